# revision 1
# baseline (speedup 1.0000x reference)
"""Temporal attention kernel, data-parallel over the hw axis on 8 NeuronCores.

Shapes (hardcoded per the problem spec):
  x                  [4, 1024, 16, 512] f32
  pos_bias           [8, 16, 16]        f32
  focus_present_mask [4]                bool
  Wqkv               [512, 1536]        f32
  Wout               [512, 512]         f32
  out                [4, 1024, 16, 512] f32

Attention is independent per (b, hw) sequence, so hw=1024 is split into 8
shards of 128; weights/bias/mask are replicated on every core.
"""

import numpy as np

HEADS = 8
N_CORES = 8


def _attn_block(x, pos_bias, focus_present_mask, Wqkv, Wout):
    import jax
    import jax.numpy as jnp

    b, hw, n, dim = x.shape
    qkv = x @ Wqkv
    q, k, v = jnp.split(qkv, 3, axis=-1)

    def split_heads(t):  # [..., n, h*d] -> [..., h, n, d]
        return t.reshape(*t.shape[:-1], HEADS, -1).swapaxes(-2, -3)

    q, k, v = split_heads(q), split_heads(k), split_heads(v)
    scale = q.shape[-1] ** -0.5
    q = q * scale
    sim = jnp.einsum('bshid,bshjd->bshij', q, k)
    sim = sim + pos_bias
    eye = jnp.eye(n, dtype=bool)
    all_true = jnp.ones((n, n), dtype=bool)
    mask = jnp.where(focus_present_mask[:, None, None, None, None],
                     eye[None, None, None], all_true[None, None, None])
    sim = jnp.where(mask, sim, -jnp.finfo(sim.dtype).max)
    attn = jax.nn.softmax(sim.astype(jnp.float32), axis=-1).astype(v.dtype)
    out = jnp.einsum('bshij,bshjd->bshid', attn, v)
    out = out.swapaxes(-2, -3).reshape(b, hw, n, -1)
    return out @ Wout


def _run_pmap(x, pos_bias, focus_present_mask, Wqkv, Wout):
    import jax

    b, hw, n, dim = x.shape
    chunk = hw // N_CORES
    # [b, hw, n, d] -> [cores, b, hw/8, n, d]
    xs = x.reshape(b, N_CORES, chunk, n, dim).transpose(1, 0, 2, 3, 4)
    xs = np.ascontiguousarray(xs)
    fn = jax.pmap(_attn_block, axis_name='i',
                  in_axes=(0, None, None, None, None),
                  devices=jax.devices()[:N_CORES])
    out = fn(xs, pos_bias, focus_present_mask, Wqkv, Wout)
    out = np.asarray(out)  # [cores, b, chunk, n, dim]
    out = out.transpose(1, 0, 2, 3, 4).reshape(b, hw, n, dim)
    return np.ascontiguousarray(out)


def _run_numpy(x, pos_bias, focus_present_mask, Wqkv, Wout):
    b, hw, n, dim = x.shape
    hidden = Wqkv.shape[1] // 3
    dh = hidden // HEADS
    qkv = x.reshape(-1, dim) @ Wqkv  # [b*hw*n, 3*hidden]
    qkv = qkv.reshape(b, hw, n, 3 * hidden)
    q, k, v = qkv[..., :hidden], qkv[..., hidden:2 * hidden], qkv[..., 2 * hidden:]

    def split_heads(t):
        return t.reshape(b, hw, n, HEADS, dh).swapaxes(-2, -3)  # [b,hw,h,n,dh]

    q, k, v = split_heads(q), split_heads(k), split_heads(v)
    q = q * (dh ** -0.5)
    sim = np.einsum('bshid,bshjd->bshij', q, k, optimize=True)
    sim = sim + pos_bias  # [h, n, n] broadcast
    eye = np.eye(n, dtype=bool)
    neg = -np.finfo(np.float32).max
    for bi in range(b):
        if focus_present_mask[bi]:
            sim[bi, :, :, ~eye] = neg
    m = sim.max(axis=-1, keepdims=True)
    e = np.exp(sim - m)
    attn = e / e.sum(axis=-1, keepdims=True)
    out = np.einsum('bshij,bshjd->bshid', attn.astype(np.float32), v, optimize=True)
    out = out.swapaxes(-2, -3).reshape(b, hw, n, hidden)
    return (out.reshape(-1, hidden) @ Wout).reshape(b, hw, n, dim).astype(np.float32)


def kernel(x, pos_bias, focus_present_mask, Wqkv, Wout):
    x = np.asarray(x, dtype=np.float32)
    pos_bias = np.asarray(pos_bias, dtype=np.float32)
    focus_present_mask = np.asarray(focus_present_mask)
    Wqkv = np.asarray(Wqkv, dtype=np.float32)
    Wout = np.asarray(Wout, dtype=np.float32)
    try:
        import jax
        if len(jax.devices()) >= N_CORES:
            return _run_pmap(x, pos_bias, focus_present_mask, Wqkv, Wout)
    except Exception:
        pass
    return _run_numpy(x, pos_bias, focus_present_mask, Wqkv, Wout)
